# revision 50
# baseline (speedup 1.0000x reference)
"""ResNet bottleneck block (1x1 -> 3x3 -> 1x1 convs, folded BN, residual ReLU)
on 8 Trainium2 NeuronCores, data-parallel over the batch dim.

Layout strategy (per core, 8 images):
  - x arranged [img, p, kc, hw] so each image DMA is one contiguous
    [128, 8*784] transfer; channel c = kc*128 + p.
  - conv1 runs in bf16 (weights folded+cast host-side); conv2 and conv3 run
    as fp8e4 DoubleRow matmuls (2 fp8 weights per PE cell -> K=256 per pass,
    ~2x ALU rate). Weights are scaled host-side into fp8's normal range
    (w2 x256, w3 x128) and activations stored 4x-scaled in fp8; the inverse
    scales fold into the PSUM-evacuation ops (activation scale= / the 512x
    residual trick below), so no extra instructions are spent descaling.
  - 1x1 convs are matmuls over the flattened spatial dim (N split 2x392).
  - 3x3 conv reads a zero-padded 30x30 (stride-912) fp8 image as ONE
    contiguous 420-column span per (dy,dx) tap: output positions live in
    padded coords, the 2 garbage columns per row are simply never
    evacuated. 9 taps x 1 DoubleRow matmul (K=256) accumulate in PSUM;
    the two 14-row halves share each tap's weights back-to-back to
    amortize the (slower, non-FWL) DoubleRow LDWEIGHTS.
  - conv3: one DoubleRow matmul per (half, out-chunk) into a 2-bank PSUM
    tile; both halves evacuate in ONE [P, 2, 392] op. The residual add is
    a scalar_tensor_tensor computing 512*x + psum (psum is 512-scaled by
    the fp8 weight scaling), then ReLU+shift: on ScalarE as
    relu(tt/512 + sh3) for 6 of 8 chunks, on VectorE as relu(tt + 512*sh3)
    (leaving the output 512x scaled, divided out host-side) for chunks
    {3,7} to balance the two evacuation engines.
  - Output stored bf16 and upcast to f32 host-side (halves the store DMA).
  - Software pipeline over images: DMA(t) / conv1(t-1) / conv2(t-2) /
    conv3+store(t-3) so the PE stream never waits on a same-image epilogue.
"""

import math
import os

import numpy as np
import ml_dtypes

import concourse.bass as bass
import concourse.mybir as mybir
import concourse.tile as tile
from concourse.bass_utils import run_bass_kernel_spmd

# Problem constants (hardcoded per the grading contract).
B, CIN, H, W = 64, 1024, 28, 28
WIDTH, COUT = 256, 1024
NCORES = 8
BPC = B // NCORES          # images per core
S = H * W                  # 784
PW = W + 2                 # 30 (padded row width)
PS = PW * PW               # 900
PSP = 912                  # padded plane stride (16-byte aligned, 12B tail)
NROW = H // 2              # 14 rows per spatial chunk
NS = NROW * W              # 392 columns per 1x1 matmul
N2 = NROW * PW             # 420 columns per conv2 DoubleRow matmul
P = 128
KC_IN = CIN // P           # 8
MC_W = WIDTH // P          # 2
MC_OUT = COUT // P         # 8
EPS = 1e-5

# fp8 scale plan: x8 = AX*x, w1' = B1*w1f, a1' = A1*a1, a2' = A2*a2,
# w2' = B2*w2f, w3' = (B3/A2)*w3f.
# psum1 = AX*B1*conv1 ; psum2 = A1*B2*conv2 ; psum3 = B3*conv3.
AX = 8.0
B1 = 128.0
A1 = 4.0
A2 = 4.0
B2 = 256.0
B3 = 512.0
# out-chunks whose conv3 evacuation leaves the output B3-scaled (divided out
# host-side). Empty: all evacuations descale on ScalarE via activation scale.
SCALED_MC = ()
# out-chunks whose residual-add runs on the PE (identity matmul) instead of
# the DVE stt. These are the groups injected at the tightest weave slots
# (~1us behind their predecessor): dropping their stt gives the DVE chain
# that recycles the 2-deep conv3 PSUM rotation ~2us between stts, at the
# cost of 2 extra 163ns identity matmuls per group.
IDENT_MC = (1, 5)

F32 = mybir.dt.float32
BF16 = mybir.dt.bfloat16
F8 = mybir.dt.float8e4
Relu = mybir.ActivationFunctionType.Relu
ADD = mybir.AluOpType.add
MAX = mybir.AluOpType.max
MULT = mybir.AluOpType.mult
DR = mybir.MatmulPerfMode.DoubleRow

BF16_NP = np.dtype(ml_dtypes.bfloat16)
F8_NP = np.dtype(ml_dtypes.float8_e4m3)
MM_MODE = "dr123"  # informational; test.py prints this

_NC_CACHE = {}
LAST_RESULT = None  # test.py reads exec_time_ns off this


def _split_multi_waits(nc, maxw=1):
    """walrus codegen rejects instructions carrying more than a couple of
    sem waits ("Too many sync wait commands"); hoist excess waits onto
    same-engine NOPs emitted just before the instruction."""
    for f in nc.m.functions:
        for blk in f.blocks:
            out = []
            changed = False
            for inst in blk.instructions:
                si = inst.sync_info
                if si is not None and len(si.on_wait) > maxw:
                    waits = list(si.on_wait)
                    head, keep = waits[:-maxw], waits[-maxw:]
                    for i in range(0, len(head), maxw):
                        nop = mybir.InstNoOp(
                            name=f"{inst.name}_waitsplit_{i}", ins=[], outs=[]
                        )
                        nop.engine = inst.engine
                        nop.sync_info = mybir.SyncInfo(
                            on_wait=head[i:i + maxw], on_update=[]
                        )
                        out.append(nop)
                    inst.sync_info = mybir.SyncInfo(
                        on_wait=keep, on_update=list(si.on_update)
                    )
                    changed = True
                out.append(inst)
            if changed:
                blk.instructions = out


def _build_nc():
    nc = bass.Bass()
    x_d = nc.dram_tensor("x", [BPC, P, KC_IN, S], BF16, kind="ExternalInput")
    x8_d = nc.dram_tensor("x8", [BPC, P, KC_IN, S], F8, kind="ExternalInput")
    w1_d = nc.dram_tensor("w1", [P, KC_IN, MC_W, P], F8, kind="ExternalInput")
    w2_d = nc.dram_tensor("w2", [P, 9, MC_W, MC_W, P], F8, kind="ExternalInput")
    w3_d = nc.dram_tensor("w3", [P, MC_W, MC_OUT, P], F8, kind="ExternalInput")
    s1_d = nc.dram_tensor("s1", [P, MC_W], F32, kind="ExternalInput")
    s2_d = nc.dram_tensor("s2", [P, MC_W], F32, kind="ExternalInput")
    s3_d = nc.dram_tensor("s3", [P, MC_OUT], F32, kind="ExternalInput")
    s3b_d = nc.dram_tensor("s3b", [P, MC_OUT], F32, kind="ExternalInput")
    id_d = nc.dram_tensor("ident", [P, P], BF16, kind="ExternalInput")
    o_d = nc.dram_tensor("o", [BPC, MC_OUT, P, S], BF16, kind="ExternalOutput")

    with tile.TileContext(nc) as tc:
        with (
            tc.tile_pool(name="consts", bufs=1) as cpool,
            tc.tile_pool(name="xin", bufs=4) as xpool,
            tc.tile_pool(name="x8in", bufs=3) as xpool8,
            tc.tile_pool(name="a1p", bufs=3) as a1pool,
            tc.tile_pool(name="a2p", bufs=2) as a2pool,
            tc.tile_pool(name="otp", bufs=18) as opool,
            tc.tile_pool(name="ttp", bufs=8) as tpool,
            tc.tile_pool(name="psp", bufs=4, space="PSUM") as pspool,
            tc.tile_pool(name="pp2", bufs=2, space="PSUM") as pppool,
        ):
            w1_sb = cpool.tile([P, KC_IN, MC_W, P], F8, tag="w1")
            w2_sb = cpool.tile([P, 9, MC_W, MC_W, P], F8, tag="w2")
            w3_sb = cpool.tile([P, MC_W, MC_OUT, P], F8, tag="w3")
            s1_sb = cpool.tile([P, MC_W], F32, tag="s1")
            s2_sb = cpool.tile([P, MC_W], F32, tag="s2")
            s3_sb = cpool.tile([P, MC_OUT], F32, tag="s3")
            s3b_sb = cpool.tile([P, MC_OUT], F32, tag="s3b")
            # Pre-warm the PE during the DMA lead-in: HAM starts the PE
            # throttled at 1.2 GHz and needs ~3.4us of sustained activity to
            # un-gate; dummy matmuls (no DMA dependency) get that out of the
            # way before the first real matmul's operands land.
            warm_sb = cpool.tile([P, P, 4], BF16, tag="warm")
            nc.vector.memset(warm_sb[:], 0.0)
            ident_sb = cpool.tile([P, P], BF16, tag="ident")
            # Trigger ScalarE's Relu ACT_TABLE_LOAD (~1.3us) during the DMA
            # lead-in instead of on conv1(0)'s first evacuation.
            actwarm = cpool.tile([P, 1], BF16, tag="actwarm")
            nc.scalar.activation(actwarm[:], warm_sb[:, 0, 0:1], Relu)
            for _ in range(56):
                wps = pspool.tile([P, N2], F32, tag="ps", name="wps")
                nc.tensor.matmul(wps[:, :64], warm_sb[:, :, 0],
                                 warm_sb[:, :64, 0], start=True, stop=True)
            # a few wide warmups bridge until the first operands land so the
            # first real matmuls run at the un-throttled clock
            for _ in range(2):
                wps = pspool.tile([P, N2], F32, tag="ps", name="wps")
                nc.tensor.matmul(wps[:, :NS], warm_sb[:, :, 0],
                                 warm_sb.rearrange("p a b -> p (a b)")[:, :NS],
                                 start=True, stop=True)

            xs = {}      # t -> bf16 [P, KC_IN, S] tile (residual)
            xs8 = {}     # t -> fp8 [P, KC_IN, S] tile (AX-scaled, conv1 rhs)
            a1s = {}     # t -> padded act1 [P, MC_W, PSP] fp8 (A1-scaled)
            a2s = {}     # t -> act2 [P, MC_W, S] fp8 (A2-scaled)

            def load(t):
                # whole-image DMAs: 12.5KB/6.3KB contiguous per-partition
                # lines give near-peak HBM packets, and the pipeline gives
                # them a full image of slack so arrival pacing doesn't
                # matter. The fp8 copy (conv1 rhs) is needed first; the bf16
                # copy is only read by conv3, two slots later.
                x8f = xpool8.tile([P, KC_IN, S], F8, tag="x8f")
                nc.sync.dma_start(x8f[:], x8_d[t])
                xs8[t] = x8f
                xf = xpool.tile([P, KC_IN, S], BF16, tag="xf")
                nc.sync.dma_start(xf[:], x_d[t])
                xs[t] = xf
                prep_a1(t)

            def prep_a1(t):
                # Allocate image t's padded conv1 output and zero its borders
                # HERE -- an image-slot before conv1(t) runs -- so the DVE
                # memsets queue in the slack window between image boundaries
                # instead of delaying the tightly-scheduled conv3 stt chain.
                a1 = a1pool.tile([P, MC_W, PSP], F8, tag="a1", name="a1")
                a14 = a1[:, :, :PS].rearrange("p m (r c) -> p m r c", c=PW)
                for mc in range(MC_W):
                    nc.vector.memset(a14[:, mc, 0:PW:PW - 1, :], 0.0)
                    nc.vector.memset(a14[:, mc, 1:PW - 1, 0:PW:PW - 1], 0.0)
                    nc.vector.memset(a1[:, mc, PS:PSP], 0.0)
                a1s[t] = a1

            def conv1(t):
                # 1x1 conv over K=1024 as 4 DoubleRow matmuls per group; the
                # two 14-row halves share each kc-pair's weights back-to-back
                # (two PSUM banks accumulate concurrently) to amortize the
                # slow DoubleRow LDWEIGHTS.
                a1 = a1s[t]
                a14 = a1[:, :, :PS].rearrange("p m (r c) -> p m r c", c=PW)
                xr = xs8[t]
                for mc in range(MC_W):
                    pss = (
                        pspool.tile([P, N2], F32, tag="ps", name="c1ps0"),
                        pspool.tile([P, N2], F32, tag="ps", name="c1ps1"),
                    )
                    for kp in range(KC_IN // 2):
                        for sc in range(2):
                            nc.tensor.matmul(
                                pss[sc][:, :NS],
                                w1_sb[:, 2 * kp:2 * kp + 2, mc, :],
                                xr[:, 2 * kp:2 * kp + 2, sc * NS:(sc + 1) * NS],
                                start=(kp == 0),
                                stop=(kp == KC_IN // 2 - 1),
                                perf_mode=DR,
                            )
                        if kp == 1:
                            yield  # mid-block conv3 injection slot
                    for sc in range(2):
                        r0 = sc * NROW
                        psr = pss[sc][:, :NS].rearrange("p (r c) -> p r c", c=W)
                        nc.scalar.activation(
                            a14[:, mc, 1 + r0:1 + r0 + NROW, 1:1 + W],
                            psr,
                            Relu,
                            scale=A1 / (AX * B1),
                            bias=s1_sb[:, mc:mc + 1],
                        )
                        yield
                del xs8[t]

            def conv2(t):
                # 3x3 conv: per out-chunk, two PSUM banks (one per 14-row
                # half) accumulate 9 DoubleRow taps; tap weights are shared
                # by the two halves back-to-back so the slow (non-FWL)
                # DoubleRow LDWEIGHTS amortizes over 2 matmuls.
                a2 = a2pool.tile([P, MC_W, S], F8, tag="a2")
                a2s[t] = a2
                a1 = a1s[t]
                for mc in range(MC_W):
                    pss = (
                        pspool.tile([P, N2], F32, tag="ps", name="ps_sc0"),
                        pspool.tile([P, N2], F32, tag="ps", name="ps_sc1"),
                    )
                    for d in range(9):
                        dy, dx = d // 3, d % 3
                        off = dy * PW + dx
                        for sc in range(2):
                            base = sc * N2 + off
                            nc.tensor.matmul(
                                pss[sc][:],
                                w2_sb[:, d, :, mc, :],
                                a1[:, :, base:base + N2],
                                start=(d == 0),
                                stop=(d == 8),
                                perf_mode=DR,
                            )
                        if d in (2, 5):
                            yield  # mid-block conv3 injection slots
                    for sc in range(2):
                        psr = pss[sc].rearrange(
                            "p (r c) -> p r c", c=PW)[:, :, :W]
                        a2r = a2[:, mc, sc * NS:(sc + 1) * NS].rearrange(
                            "p (r c) -> p r c", c=W)
                        nc.scalar.activation(
                            a2r,
                            psr,
                            Relu,
                            scale=A2 / (A1 * B2),
                            bias=s2_sb[:, mc:mc + 1],
                        )
                        if sc == 1 and mc == MC_W - 1:
                            del a1s[t]
                        yield

            def conv3(t, last=False):
                # One DoubleRow matmul per (half, out-chunk) into a 2-bank
                # PSUM tile; both halves evacuate in single [P, 2, 392] ops.
                # Residual: tt = 512*x + psum on VectorE, then ReLU+shift on
                # ScalarE (6/8 chunks) or VectorE 512-scaled (chunks 3,7;
                # divided out host-side) to balance the evac engines. The
                # last image folds the residual into the PE instead (identity
                # matmul) so the drain tail has no VectorE stt dependency.
                a2r = a2s[t]
                xf = xs[t]
                osbs = [
                    opool.tile([P, S], BF16, tag="osb", name="osb")
                    for _ in range(MC_OUT)
                ]
                for mc in range(MC_OUT):
                    osb = osbs[mc]
                    use_ident = last or mc in IDENT_MC
                    pp = pppool.tile([P, 1024], F32, tag="pp")
                    for sc in range(2):
                        nc.tensor.matmul(
                            pp[:, sc * 512:sc * 512 + NS],
                            w3_sb[:, :, mc, :],
                            a2r[:, :, sc * NS:(sc + 1) * NS],
                            start=True,
                            stop=not use_ident,
                            perf_mode=DR,
                        )
                        if use_ident:
                            nc.tensor.matmul(
                                pp[:, sc * 512:sc * 512 + NS],
                                ident_sb[:],
                                xf[:, mc, sc * NS:(sc + 1) * NS],
                                start=False,
                                stop=True,
                            )
                    ppv = pp.rearrange("p (s n) -> p s n", n=512)[:, :, :NS]
                    osbv = osb.rearrange("p (s n) -> p s n", n=NS)
                    if use_ident:
                        src = ppv
                    else:
                        tt = tpool.tile([P, S], BF16, tag="tt")
                        ttv = tt.rearrange("p (s n) -> p s n", n=NS)
                        xfv = xf[:, mc].rearrange("p (s n) -> p s n", n=NS)
                        nc.vector.scalar_tensor_tensor(
                            ttv, xfv, B3, ppv, MULT, ADD
                        )
                        src = ttv
                    # All conv3 evacuations run on ScalarE: the DVE carries
                    # ONLY the stt chain, whose completion gates the 2-deep
                    # conv3 PSUM rotation with ~64ns of margin -- any other
                    # DVE op queued among the stts stalls the PE.
                    nc.scalar.activation(
                        osbv, src, Relu,
                        scale=1.0 / B3,
                        bias=s3_sb[:, mc:mc + 1],
                    )
                    # Output pushes stay on the Sync queue: issuing them from
                    # ScalarE would serialize ~600ns of descriptor issue into
                    # the activation chain that recycles every PSUM group.
                    # Exception: the drain (last image) alternates queues so
                    # the final 8 pushes don't serialize behind one queue.
                    if last and mc % 2 == 0:
                        nc.scalar.dma_start(o_d[t, mc], osb[:])
                    else:
                        nc.sync.dma_start(o_d[t, mc], osb[:])
                    yield
                del a2s[t], xs[t]

            # Startup DMAs: each queue-side dma_start costs ~600ns of SERIAL
            # issue time on its engine queue, so a single-queue startup burns
            # >10us before conv1(0)'s data even lands. Spread the issues
            # across the four engine queues that are idle during the lead-in
            # (Sync keeps conv1(0)'s critical path; Scalar gets conv2's
            # weights; Vector/GpSimd prefetch the rest).
            # Single-queue startup: the Sync queue issues serially (~600ns
            # per descriptor) which is slower to ISSUE than spreading across
            # queues, but it keeps the HBM transfers strictly in priority
            # order -- parallel-queue variants measured WORSE because the
            # bulk prefetches stole bandwidth from conv1(0)'s critical x8.
            x80 = xpool8.tile([P, KC_IN, S], F8, tag="x8f", name="x8f")
            nc.sync.dma_start(w1_sb[:, 0:2], w1_d[:, 0:2])
            nc.sync.dma_start(x80[:, 0:2], x8_d[0, :, 0:2])
            nc.sync.dma_start(s1_sb[:], s1_d[:])
            nc.sync.dma_start(w1_sb[:, 2:8], w1_d[:, 2:8])
            nc.sync.dma_start(x80[:, 2:4], x8_d[0, :, 2:4])
            nc.sync.dma_start(x80[:, 4:6], x8_d[0, :, 4:6])
            nc.sync.dma_start(x80[:, 6:8], x8_d[0, :, 6:8])
            xs8[0] = x80
            nc.sync.dma_start(s2_sb[:], s2_d[:])
            for i3 in range(3):
                nc.sync.dma_start(
                    w2_sb[:, 3 * i3:3 * i3 + 3],
                    w2_d[:, 3 * i3:3 * i3 + 3],
                )
            x81 = xpool8.tile([P, KC_IN, S], F8, tag="x8f", name="x8f1")
            nc.sync.dma_start(x81[:], x8_d[1])
            xs8[1] = x81
            xf0 = xpool.tile([P, KC_IN, S], BF16, tag="xf", name="xf")
            nc.sync.dma_start(xf0[:], x_d[0])
            xs[0] = xf0
            xf1 = xpool.tile([P, KC_IN, S], BF16, tag="xf", name="xf1")
            nc.sync.dma_start(xf1[:], x_d[1])
            xs[1] = xf1
            prep_a1(0)
            prep_a1(1)
            nc.sync.dma_start(w3_sb[:], w3_d[:])
            nc.sync.dma_start(s3_sb[:], s3_d[:])
            nc.sync.dma_start(s3b_sb[:], s3b_d[:])
            nc.sync.dma_start(ident_sb[:], id_d[:])

            # conv2(t-2) groups are issued before conv1(t-1) groups: during
            # the lead-in conv2(0)'s weights arrive well before conv1(1)'s
            # image, so this order keeps the PE fed (and HAM un-throttled)
            # through the fill. conv3(t-3)'s groups are woven 1-per-group
            # between them: conv3's matmul bursts are short relative to their
            # evacuations, so run back-to-back they stall the PE on PSUM
            # recycling; interleaved they never wait.
            def weave(big, small, ratio=1):
                # ratio may be an int (uniform) or a per-yield list: conv3
                # groups are injected right after big-generator yields, so
                # the list shapes WHERE the small groups land -- front-loaded
                # onto the conv2 yields (1.6us of PE work behind each) and
                # one per conv1 yield, keeping the group-issue rate below the
                # DVE stt service rate that recycles conv3's PSUM pair.
                y = 0
                for g in big:
                    for _ in g:
                        r = ratio[y] if isinstance(ratio, list) else ratio
                        y += 1
                        for _ in range(r):
                            if small is not None and next(small, "end") == "end":
                                small = None
                return small

            def chain(*gens):
                for g in gens:
                    yield from g

            for t in range(BPC + 2):
                if 1 < t < BPC:
                    load(t)
                big = []
                if 0 <= t - 2 < BPC:
                    big.append(conv2(t - 2))
                if 0 <= t - 1 < BPC:
                    big.append(conv1(t - 1))
                if t == BPC + 1:
                    # final weave: hide 4 of conv3(last-1)'s groups behind
                    # conv2(last)'s yields -- [2,0,2,0] keeps the pairs at
                    # the block boundaries (adjacent evac yields carry no PE
                    # work, so ratio=2 everywhere would make a 4-burst that
                    # outruns the stt chain). The last image's conv3 may only
                    # start AFTER conv2(last) has evacuated all four a2
                    # quarters (every conv3 DoubleRow matmul reads BOTH kc
                    # planes of a2; issuing early corrupts it), so the drain
                    # interleaves the leftover stt-paced conv3(last-1) groups
                    # with the act-paced (stt-free) conv3(last) groups --
                    # each provides the PE work that hides the other's
                    # evacuation latency.
                    rest = weave(
                        big, conv3(t - 3),
                        ratio=[1, 1, 1, 0, 1, 1, 1, 0],
                    )
                    lastg = conv3(t - 2, last=True)
                    alive = True
                    while alive:
                        alive = False
                        if rest is not None and next(rest, "end") != "end":
                            alive = True
                        if next(lastg, "end") != "end":
                            alive = True
                else:
                    small = conv3(t - 3) if 0 <= t - 3 < BPC else None
                    # Single conv3 groups at the evenly-spaced injection
                    # slots (conv2: after taps 2/5 and at the first evac of
                    # each block; conv1: mid-block and first evac): every
                    # consecutive pair of groups is >=1us of PE work apart,
                    # so the DVE stt chain (965ns/group), which recycles the
                    # 2-deep conv3 PSUM rotation, never falls behind.
                    rest = weave(
                        big, small,
                        ratio=[1, 1, 1, 0, 1, 1, 1, 0, 1, 1, 0, 0, 0, 0],
                    )
                    if rest is not None:
                        for _ in rest:
                            pass

    return nc


def _prep_inputs(x, w1, w2, w3, g1, b1, m1, v1, g2, b2, m2, v2, g3, b3, m3, v3):
    """Fold BN into weights/shifts and pack everything into per-core maps."""

    def fold(wv, g, bb, m, v):
        inv = (g / np.sqrt(v + EPS)).astype(np.float32)
        shift = (bb - m * inv).astype(np.float32)
        return np.asarray(wv, np.float32) * inv[:, None, None, None], shift

    def to_f8(a):
        return np.clip(a, -240.0, 240.0).astype(F8_NP)

    w1f, sh1 = fold(np.asarray(w1, np.float32), g1, b1, m1, v1)
    w2f, sh2 = fold(np.asarray(w2, np.float32), g2, b2, m2, v2)
    w3f, sh3 = fold(np.asarray(w3, np.float32), g3, b3, m3, v3)

    # lhsT layouts: partition = K-within-chunk, free = [kc?, d?, mc, m]
    w1h = to_f8(np.ascontiguousarray(
        (B1 * w1f)[:, :, 0, 0].T.reshape(KC_IN, P, MC_W, P).transpose(1, 0, 2, 3)
    ))
    w2h = to_f8(np.ascontiguousarray(
        (B2 * w2f).transpose(2, 3, 1, 0)    # [ky, kx, in, out]
        .reshape(9, MC_W, P, MC_W, P)
        .transpose(2, 0, 1, 3, 4)
    ))
    w3h = to_f8(np.ascontiguousarray(
        ((B3 / A2) * w3f)[:, :, 0, 0].T
        .reshape(MC_W, P, MC_OUT, P).transpose(1, 0, 2, 3)
    ))
    s1h = np.ascontiguousarray((A1 * sh1).reshape(MC_W, P).T)
    s2h = np.ascontiguousarray((A2 * sh2).reshape(MC_W, P).T)
    s3h = np.ascontiguousarray(sh3.reshape(MC_OUT, P).T)
    s3bh = np.ascontiguousarray((B3 * sh3).reshape(MC_OUT, P).T)

    xf32 = np.asarray(x, np.float32)
    xnp = xf32.astype(BF16_NP)
    x8np = to_f8(AX * xf32)
    in_maps = []
    for c in range(NCORES):
        sl = slice(c * BPC, (c + 1) * BPC)
        xc = np.ascontiguousarray(
            xnp[sl].reshape(BPC, KC_IN, P, S).transpose(0, 2, 1, 3)
        )
        x8c = np.ascontiguousarray(
            x8np[sl].reshape(BPC, KC_IN, P, S).transpose(0, 2, 1, 3)
        )
        in_maps.append({
            "x": xc, "x8": x8c, "w1": w1h, "w2": w2h, "w3": w3h,
            "s1": s1h, "s2": s2h, "s3": s3h, "s3b": s3bh,
            "ident": (B3 * np.eye(P)).astype(BF16_NP),
        })
    return in_maps


def _ensure_ntff_hook():
    """If tracing is requested but this image's antenv lacks axon_hooks,
    register an in-process shim (or disable tracing) so run_bass_kernel_spmd
    doesn't crash on the import."""
    if os.environ.get("BASS_TRACE") != "1":
        return
    try:
        import antenv.axon_hooks  # noqa: F401
        return
    except ImportError:
        pass
    try:
        import sys
        import types
        import antenv
        from trn_agent_boot.trn_boot import _ntff_profile_via_ctypes

        hook = _ntff_profile_via_ctypes("/opt/axon/libaxon_pjrt.so")
        mod = types.ModuleType("antenv.axon_hooks")
        state = {"hook": hook}
        mod.set_axon_ntff_profile_hook = lambda h: state.__setitem__("hook", h)
        mod.get_axon_ntff_profile_hook = lambda: state["hook"]
        antenv.axon_hooks = mod
        sys.modules["antenv.axon_hooks"] = mod
    except Exception:
        os.environ["BASS_NEVER_TRACE"] = "1"


def kernel(**inputs):
    global LAST_RESULT
    _ensure_ntff_hook()
    if "nc" not in _NC_CACHE:
        nc = _build_nc()
        _split_multi_waits(nc)  # HW-only legalization; CoreSim can't run it
        _NC_CACHE["nc"] = nc
    nc = _NC_CACHE["nc"]
    in_maps = _prep_inputs(**inputs)
    res = run_bass_kernel_spmd(nc, in_maps, list(range(NCORES)))
    LAST_RESULT = res
    outs = []
    for r in res.results:
        of = r["o"].astype(np.float32)      # [BPC, MC_OUT, P, S]
        of[:, list(SCALED_MC)] /= B3        # undo the VectorE-path scaling
        outs.append(of)
    out = np.concatenate(outs, axis=0)
    return np.ascontiguousarray(out.reshape(B, COUT, H, W))


# revision 51
# speedup vs baseline: 1.0912x; 1.0912x over previous
"""ResNet bottleneck block (1x1 -> 3x3 -> 1x1 convs, folded BN, residual ReLU)
on 8 Trainium2 NeuronCores, data-parallel over the batch dim.

Layout strategy (per core, 8 images):
  - x arranged [img, p, kc, hw] so each image DMA is one contiguous
    [128, 8*784] transfer; channel c = kc*128 + p.
  - conv1 runs in bf16 (weights folded+cast host-side); conv2 and conv3 run
    as fp8e4 DoubleRow matmuls (2 fp8 weights per PE cell -> K=256 per pass,
    ~2x ALU rate). Weights are scaled host-side into fp8's normal range
    (w2 x256, w3 x128) and activations stored 4x-scaled in fp8; the inverse
    scales fold into the PSUM-evacuation ops (activation scale= / the 512x
    residual trick below), so no extra instructions are spent descaling.
  - 1x1 convs are matmuls over the flattened spatial dim (N split 2x392).
  - 3x3 conv reads a zero-padded 30x30 (stride-912) fp8 image as ONE
    contiguous 420-column span per (dy,dx) tap: output positions live in
    padded coords, the 2 garbage columns per row are simply never
    evacuated. 9 taps x 1 DoubleRow matmul (K=256) accumulate in PSUM;
    the two 14-row halves share each tap's weights back-to-back to
    amortize the (slower, non-FWL) DoubleRow LDWEIGHTS.
  - conv3: one DoubleRow matmul per (half, out-chunk) into a 2-bank PSUM
    tile; both halves evacuate in ONE [P, 2, 392] op. The residual add is
    a scalar_tensor_tensor computing 512*x + psum (psum is 512-scaled by
    the fp8 weight scaling), then ReLU+shift: on ScalarE as
    relu(tt/512 + sh3) for 6 of 8 chunks, on VectorE as relu(tt + 512*sh3)
    (leaving the output 512x scaled, divided out host-side) for chunks
    {3,7} to balance the two evacuation engines.
  - Output stored bf16 and upcast to f32 host-side (halves the store DMA).
  - Software pipeline over images: DMA(t) / conv1(t-1) / conv2(t-2) /
    conv3+store(t-3) so the PE stream never waits on a same-image epilogue.
"""

import math
import os

import numpy as np
import ml_dtypes

import concourse.bass as bass
import concourse.mybir as mybir
import concourse.tile as tile
from concourse.bass_utils import run_bass_kernel_spmd

# Problem constants (hardcoded per the grading contract).
B, CIN, H, W = 64, 1024, 28, 28
WIDTH, COUT = 256, 1024
NCORES = 8
BPC = B // NCORES          # images per core
S = H * W                  # 784
PW = W + 2                 # 30 (padded row width)
PS = PW * PW               # 900
PSP = 912                  # padded plane stride (16-byte aligned, 12B tail)
NROW = H // 2              # 14 rows per spatial chunk
NS = NROW * W              # 392 columns per 1x1 matmul
N2 = NROW * PW             # 420 columns per conv2 DoubleRow matmul
P = 128
KC_IN = CIN // P           # 8
MC_W = WIDTH // P          # 2
MC_OUT = COUT // P         # 8
EPS = 1e-5

# fp8 scale plan: x8 = AX*x, w1' = B1*w1f, a1' = A1*a1, a2' = A2*a2,
# w2' = B2*w2f, w3' = (B3/A2)*w3f.
# psum1 = AX*B1*conv1 ; psum2 = A1*B2*conv2 ; psum3 = B3*conv3.
AX = 8.0
B1 = 128.0
A1 = 4.0
A2 = 4.0
B2 = 256.0
B3 = 512.0
# out-chunks whose conv3 evacuation leaves the output B3-scaled (divided out
# host-side). Empty: all evacuations descale on ScalarE via activation scale.
SCALED_MC = ()
# out-chunks whose residual-add runs on the PE (identity matmul) instead of
# the DVE stt. Empty: interleaving bf16 identity matmuls into the fp8
# DoubleRow stream measured WORSE (the stationary-mode switches disrupt the
# PE pipeline); only the drain (last image) uses the identity path, where
# there is no DoubleRow stream left to disturb.
IDENT_MC = ()

F32 = mybir.dt.float32
BF16 = mybir.dt.bfloat16
F8 = mybir.dt.float8e4
Relu = mybir.ActivationFunctionType.Relu
ADD = mybir.AluOpType.add
MAX = mybir.AluOpType.max
MULT = mybir.AluOpType.mult
DR = mybir.MatmulPerfMode.DoubleRow

BF16_NP = np.dtype(ml_dtypes.bfloat16)
F8_NP = np.dtype(ml_dtypes.float8_e4m3)
MM_MODE = "dr123"  # informational; test.py prints this

_NC_CACHE = {}
LAST_RESULT = None  # test.py reads exec_time_ns off this


def _split_multi_waits(nc, maxw=1):
    """walrus codegen rejects instructions carrying more than a couple of
    sem waits ("Too many sync wait commands"); hoist excess waits onto
    same-engine NOPs emitted just before the instruction."""
    for f in nc.m.functions:
        for blk in f.blocks:
            out = []
            changed = False
            for inst in blk.instructions:
                si = inst.sync_info
                if si is not None and len(si.on_wait) > maxw:
                    waits = list(si.on_wait)
                    head, keep = waits[:-maxw], waits[-maxw:]
                    for i in range(0, len(head), maxw):
                        nop = mybir.InstNoOp(
                            name=f"{inst.name}_waitsplit_{i}", ins=[], outs=[]
                        )
                        nop.engine = inst.engine
                        nop.sync_info = mybir.SyncInfo(
                            on_wait=head[i:i + maxw], on_update=[]
                        )
                        out.append(nop)
                    inst.sync_info = mybir.SyncInfo(
                        on_wait=keep, on_update=list(si.on_update)
                    )
                    changed = True
                out.append(inst)
            if changed:
                blk.instructions = out


def _build_nc():
    nc = bass.Bass()
    x_d = nc.dram_tensor("x", [BPC, P, KC_IN, S], BF16, kind="ExternalInput")
    x8_d = nc.dram_tensor("x8", [BPC, P, KC_IN, S], F8, kind="ExternalInput")
    w1_d = nc.dram_tensor("w1", [P, KC_IN, MC_W, P], F8, kind="ExternalInput")
    w2_d = nc.dram_tensor("w2", [P, 9, MC_W, MC_W, P], F8, kind="ExternalInput")
    w3_d = nc.dram_tensor("w3", [P, MC_W, MC_OUT, P], F8, kind="ExternalInput")
    s1_d = nc.dram_tensor("s1", [P, MC_W], F32, kind="ExternalInput")
    s2_d = nc.dram_tensor("s2", [P, MC_W], F32, kind="ExternalInput")
    s3_d = nc.dram_tensor("s3", [P, MC_OUT], F32, kind="ExternalInput")
    s3b_d = nc.dram_tensor("s3b", [P, MC_OUT], F32, kind="ExternalInput")
    id_d = nc.dram_tensor("ident", [P, P], BF16, kind="ExternalInput")
    o_d = nc.dram_tensor("o", [BPC, MC_OUT, P, S], BF16, kind="ExternalOutput")

    with tile.TileContext(nc) as tc:
        with (
            tc.tile_pool(name="consts", bufs=1) as cpool,
            tc.tile_pool(name="xin", bufs=4) as xpool,
            tc.tile_pool(name="x8in", bufs=3) as xpool8,
            tc.tile_pool(name="a1p", bufs=3) as a1pool,
            tc.tile_pool(name="a2p", bufs=2) as a2pool,
            tc.tile_pool(name="otp", bufs=18) as opool,
            tc.tile_pool(name="ttp", bufs=8) as tpool,
            tc.tile_pool(name="psp", bufs=4, space="PSUM") as pspool,
            tc.tile_pool(name="pp2", bufs=2, space="PSUM") as pppool,
        ):
            w1_sb = cpool.tile([P, KC_IN, MC_W, P], F8, tag="w1")
            w2_sb = cpool.tile([P, 9, MC_W, MC_W, P], F8, tag="w2")
            w3_sb = cpool.tile([P, MC_W, MC_OUT, P], F8, tag="w3")
            s1_sb = cpool.tile([P, MC_W], F32, tag="s1")
            s2_sb = cpool.tile([P, MC_W], F32, tag="s2")
            s3_sb = cpool.tile([P, MC_OUT], F32, tag="s3")
            s3b_sb = cpool.tile([P, MC_OUT], F32, tag="s3b")
            # Pre-warm the PE during the DMA lead-in: HAM starts the PE
            # throttled at 1.2 GHz and needs ~3.4us of sustained activity to
            # un-gate; dummy matmuls (no DMA dependency) get that out of the
            # way before the first real matmul's operands land.
            warm_sb = cpool.tile([P, P, 4], BF16, tag="warm")
            nc.vector.memset(warm_sb[:], 0.0)
            ident_sb = cpool.tile([P, P], BF16, tag="ident")
            # Trigger ScalarE's Relu ACT_TABLE_LOAD (~1.3us) during the DMA
            # lead-in instead of on conv1(0)'s first evacuation.
            actwarm = cpool.tile([P, 1], BF16, tag="actwarm")
            nc.scalar.activation(actwarm[:], warm_sb[:, 0, 0:1], Relu)
            for _ in range(56):
                wps = pspool.tile([P, N2], F32, tag="ps", name="wps")
                nc.tensor.matmul(wps[:, :64], warm_sb[:, :, 0],
                                 warm_sb[:, :64, 0], start=True, stop=True)
            # a few wide warmups bridge until the first operands land so the
            # first real matmuls run at the un-throttled clock
            for _ in range(2):
                wps = pspool.tile([P, N2], F32, tag="ps", name="wps")
                nc.tensor.matmul(wps[:, :NS], warm_sb[:, :, 0],
                                 warm_sb.rearrange("p a b -> p (a b)")[:, :NS],
                                 start=True, stop=True)

            xs = {}      # t -> bf16 [P, KC_IN, S] tile (residual)
            xs8 = {}     # t -> fp8 [P, KC_IN, S] tile (AX-scaled, conv1 rhs)
            a1s = {}     # t -> padded act1 [P, MC_W, PSP] fp8 (A1-scaled)
            a2s = {}     # t -> act2 [P, MC_W, S] fp8 (A2-scaled)

            def load(t):
                # whole-image DMAs: 12.5KB/6.3KB contiguous per-partition
                # lines give near-peak HBM packets, and the pipeline gives
                # them a full image of slack so arrival pacing doesn't
                # matter. The fp8 copy (conv1 rhs) is needed first; the bf16
                # copy is only read by conv3, two slots later.
                x8f = xpool8.tile([P, KC_IN, S], F8, tag="x8f")
                nc.sync.dma_start(x8f[:], x8_d[t])
                xs8[t] = x8f
                xf = xpool.tile([P, KC_IN, S], BF16, tag="xf")
                nc.sync.dma_start(xf[:], x_d[t])
                xs[t] = xf
                prep_a1(t)

            def prep_a1(t):
                # Allocate image t's padded conv1 output and zero its borders
                # HERE -- an image-slot before conv1(t) runs -- so the DVE
                # memsets queue in the slack window between image boundaries
                # instead of delaying the tightly-scheduled conv3 stt chain.
                a1 = a1pool.tile([P, MC_W, PSP], F8, tag="a1", name="a1")
                a14 = a1[:, :, :PS].rearrange("p m (r c) -> p m r c", c=PW)
                for mc in range(MC_W):
                    nc.vector.memset(a14[:, mc, 0:PW:PW - 1, :], 0.0)
                    nc.vector.memset(a14[:, mc, 1:PW - 1, 0:PW:PW - 1], 0.0)
                    nc.vector.memset(a1[:, mc, PS:PSP], 0.0)
                a1s[t] = a1

            def conv1(t):
                # 1x1 conv over K=1024 as 4 DoubleRow matmuls per group; the
                # two 14-row halves share each kc-pair's weights back-to-back
                # (two PSUM banks accumulate concurrently) to amortize the
                # slow DoubleRow LDWEIGHTS.
                a1 = a1s[t]
                a14 = a1[:, :, :PS].rearrange("p m (r c) -> p m r c", c=PW)
                xr = xs8[t]
                for mc in range(MC_W):
                    pss = (
                        pspool.tile([P, N2], F32, tag="ps", name="c1ps0"),
                        pspool.tile([P, N2], F32, tag="ps", name="c1ps1"),
                    )
                    for kp in range(KC_IN // 2):
                        for sc in range(2):
                            nc.tensor.matmul(
                                pss[sc][:, :NS],
                                w1_sb[:, 2 * kp:2 * kp + 2, mc, :],
                                xr[:, 2 * kp:2 * kp + 2, sc * NS:(sc + 1) * NS],
                                start=(kp == 0),
                                stop=(kp == KC_IN // 2 - 1),
                                perf_mode=DR,
                            )
                        if kp == 1:
                            yield  # mid-block conv3 injection slot
                    for sc in range(2):
                        r0 = sc * NROW
                        psr = pss[sc][:, :NS].rearrange("p (r c) -> p r c", c=W)
                        nc.scalar.activation(
                            a14[:, mc, 1 + r0:1 + r0 + NROW, 1:1 + W],
                            psr,
                            Relu,
                            scale=A1 / (AX * B1),
                            bias=s1_sb[:, mc:mc + 1],
                        )
                        yield
                del xs8[t]

            def conv2(t):
                # 3x3 conv: per out-chunk, two PSUM banks (one per 14-row
                # half) accumulate 9 DoubleRow taps; tap weights are shared
                # by the two halves back-to-back so the slow (non-FWL)
                # DoubleRow LDWEIGHTS amortizes over 2 matmuls.
                a2 = a2pool.tile([P, MC_W, S], F8, tag="a2")
                a2s[t] = a2
                a1 = a1s[t]
                for mc in range(MC_W):
                    pss = (
                        pspool.tile([P, N2], F32, tag="ps", name="ps_sc0"),
                        pspool.tile([P, N2], F32, tag="ps", name="ps_sc1"),
                    )
                    for d in range(9):
                        dy, dx = d // 3, d % 3
                        off = dy * PW + dx
                        for sc in range(2):
                            base = sc * N2 + off
                            nc.tensor.matmul(
                                pss[sc][:],
                                w2_sb[:, d, :, mc, :],
                                a1[:, :, base:base + N2],
                                start=(d == 0),
                                stop=(d == 8),
                                perf_mode=DR,
                            )
                        if d in (2, 5):
                            yield  # mid-block conv3 injection slots
                    for sc in range(2):
                        psr = pss[sc].rearrange(
                            "p (r c) -> p r c", c=PW)[:, :, :W]
                        a2r = a2[:, mc, sc * NS:(sc + 1) * NS].rearrange(
                            "p (r c) -> p r c", c=W)
                        nc.scalar.activation(
                            a2r,
                            psr,
                            Relu,
                            scale=A2 / (A1 * B2),
                            bias=s2_sb[:, mc:mc + 1],
                        )
                        if sc == 1 and mc == MC_W - 1:
                            del a1s[t]
                        yield

            def conv3(t, last=False):
                # One DoubleRow matmul per (half, out-chunk) into a 2-bank
                # PSUM tile; both halves evacuate in single [P, 2, 392] ops.
                # Residual: tt = 512*x + psum on VectorE, then ReLU+shift on
                # ScalarE (6/8 chunks) or VectorE 512-scaled (chunks 3,7;
                # divided out host-side) to balance the evac engines. The
                # last image folds the residual into the PE instead (identity
                # matmul) so the drain tail has no VectorE stt dependency.
                a2r = a2s[t]
                xf = xs[t]
                osbs = [
                    opool.tile([P, S], BF16, tag="osb", name="osb")
                    for _ in range(MC_OUT)
                ]
                for mc in range(MC_OUT):
                    osb = osbs[mc]
                    use_ident = last or mc in IDENT_MC
                    pp = pppool.tile([P, 1024], F32, tag="pp")
                    for sc in range(2):
                        nc.tensor.matmul(
                            pp[:, sc * 512:sc * 512 + NS],
                            w3_sb[:, :, mc, :],
                            a2r[:, :, sc * NS:(sc + 1) * NS],
                            start=True,
                            stop=not use_ident,
                            perf_mode=DR,
                        )
                        if use_ident:
                            nc.tensor.matmul(
                                pp[:, sc * 512:sc * 512 + NS],
                                ident_sb[:],
                                xf[:, mc, sc * NS:(sc + 1) * NS],
                                start=False,
                                stop=True,
                            )
                    ppv = pp.rearrange("p (s n) -> p s n", n=512)[:, :, :NS]
                    osbv = osb.rearrange("p (s n) -> p s n", n=NS)
                    if use_ident:
                        src = ppv
                    else:
                        tt = tpool.tile([P, S], BF16, tag="tt")
                        ttv = tt.rearrange("p (s n) -> p s n", n=NS)
                        xfv = xf[:, mc].rearrange("p (s n) -> p s n", n=NS)
                        nc.vector.scalar_tensor_tensor(
                            ttv, xfv, B3, ppv, MULT, ADD
                        )
                        src = ttv
                    # All conv3 evacuations run on ScalarE: the DVE carries
                    # ONLY the stt chain, whose completion gates the 2-deep
                    # conv3 PSUM rotation with ~64ns of margin -- any other
                    # DVE op queued among the stts stalls the PE.
                    nc.scalar.activation(
                        osbv, src, Relu,
                        scale=1.0 / B3,
                        bias=s3_sb[:, mc:mc + 1],
                    )
                    # Output pushes stay on the Sync queue: issuing them from
                    # ScalarE would serialize ~600ns of descriptor issue into
                    # the activation chain that recycles every PSUM group.
                    # Exception: the drain (last image) alternates queues so
                    # the final 8 pushes don't serialize behind one queue.
                    if last and mc % 2 == 0:
                        nc.scalar.dma_start(o_d[t, mc], osb[:])
                    else:
                        nc.sync.dma_start(o_d[t, mc], osb[:])
                    yield
                del a2s[t], xs[t]

            # Startup DMAs: each queue-side dma_start costs ~600ns of SERIAL
            # issue time on its engine queue, so a single-queue startup burns
            # >10us before conv1(0)'s data even lands. Spread the issues
            # across the four engine queues that are idle during the lead-in
            # (Sync keeps conv1(0)'s critical path; Scalar gets conv2's
            # weights; Vector/GpSimd prefetch the rest).
            # Single-queue startup: the Sync queue issues serially (~600ns
            # per descriptor) which is slower to ISSUE than spreading across
            # queues, but it keeps the HBM transfers strictly in priority
            # order -- parallel-queue variants measured WORSE because the
            # bulk prefetches stole bandwidth from conv1(0)'s critical x8.
            x80 = xpool8.tile([P, KC_IN, S], F8, tag="x8f", name="x8f")
            nc.sync.dma_start(w1_sb[:, 0:2], w1_d[:, 0:2])
            nc.sync.dma_start(x80[:, 0:2], x8_d[0, :, 0:2])
            nc.sync.dma_start(s1_sb[:], s1_d[:])
            nc.sync.dma_start(w1_sb[:, 2:8], w1_d[:, 2:8])
            nc.sync.dma_start(x80[:, 2:4], x8_d[0, :, 2:4])
            nc.sync.dma_start(x80[:, 4:6], x8_d[0, :, 4:6])
            nc.sync.dma_start(x80[:, 6:8], x8_d[0, :, 6:8])
            xs8[0] = x80
            nc.sync.dma_start(s2_sb[:], s2_d[:])
            for i3 in range(3):
                nc.sync.dma_start(
                    w2_sb[:, 3 * i3:3 * i3 + 3],
                    w2_d[:, 3 * i3:3 * i3 + 3],
                )
            x81 = xpool8.tile([P, KC_IN, S], F8, tag="x8f", name="x8f1")
            nc.sync.dma_start(x81[:], x8_d[1])
            xs8[1] = x81
            xf0 = xpool.tile([P, KC_IN, S], BF16, tag="xf", name="xf")
            nc.sync.dma_start(xf0[:], x_d[0])
            xs[0] = xf0
            xf1 = xpool.tile([P, KC_IN, S], BF16, tag="xf", name="xf1")
            nc.sync.dma_start(xf1[:], x_d[1])
            xs[1] = xf1
            prep_a1(0)
            prep_a1(1)
            nc.sync.dma_start(w3_sb[:], w3_d[:])
            nc.sync.dma_start(s3_sb[:], s3_d[:])
            nc.sync.dma_start(s3b_sb[:], s3b_d[:])
            nc.sync.dma_start(ident_sb[:], id_d[:])

            # conv2(t-2) groups are issued before conv1(t-1) groups: during
            # the lead-in conv2(0)'s weights arrive well before conv1(1)'s
            # image, so this order keeps the PE fed (and HAM un-throttled)
            # through the fill. conv3(t-3)'s groups are woven 1-per-group
            # between them: conv3's matmul bursts are short relative to their
            # evacuations, so run back-to-back they stall the PE on PSUM
            # recycling; interleaved they never wait.
            def weave(big, small, ratio=1):
                # ratio may be an int (uniform) or a per-yield list: conv3
                # groups are injected right after big-generator yields, so
                # the list shapes WHERE the small groups land -- front-loaded
                # onto the conv2 yields (1.6us of PE work behind each) and
                # one per conv1 yield, keeping the group-issue rate below the
                # DVE stt service rate that recycles conv3's PSUM pair.
                y = 0
                for g in big:
                    for _ in g:
                        r = ratio[y] if isinstance(ratio, list) else ratio
                        y += 1
                        for _ in range(r):
                            if small is not None and next(small, "end") == "end":
                                small = None
                return small

            def chain(*gens):
                for g in gens:
                    yield from g

            for t in range(BPC + 2):
                if 1 < t < BPC:
                    load(t)
                big = []
                if 0 <= t - 2 < BPC:
                    big.append(conv2(t - 2))
                if 0 <= t - 1 < BPC:
                    big.append(conv1(t - 1))
                if t == BPC + 1:
                    # final weave: hide 4 of conv3(last-1)'s groups behind
                    # conv2(last)'s yields -- [2,0,2,0] keeps the pairs at
                    # the block boundaries (adjacent evac yields carry no PE
                    # work, so ratio=2 everywhere would make a 4-burst that
                    # outruns the stt chain). The last image's conv3 may only
                    # start AFTER conv2(last) has evacuated all four a2
                    # quarters (every conv3 DoubleRow matmul reads BOTH kc
                    # planes of a2; issuing early corrupts it), so the drain
                    # interleaves the leftover stt-paced conv3(last-1) groups
                    # with the act-paced (stt-free) conv3(last) groups --
                    # each provides the PE work that hides the other's
                    # evacuation latency.
                    rest = weave(
                        big, conv3(t - 3),
                        ratio=[1, 1, 1, 0, 1, 1, 1, 0],
                    )
                    lastg = conv3(t - 2, last=True)
                    alive = True
                    while alive:
                        alive = False
                        if rest is not None and next(rest, "end") != "end":
                            alive = True
                        if next(lastg, "end") != "end":
                            alive = True
                else:
                    small = conv3(t - 3) if 0 <= t - 3 < BPC else None
                    # Single conv3 groups at the evenly-spaced injection
                    # slots (conv2: after taps 2/5 and at the first evac of
                    # each block; conv1: mid-block and first evac): every
                    # consecutive pair of groups is >=1us of PE work apart,
                    # so the DVE stt chain (965ns/group), which recycles the
                    # 2-deep conv3 PSUM rotation, never falls behind.
                    rest = weave(
                        big, small,
                        ratio=[1, 1, 1, 0, 1, 1, 1, 0, 1, 1, 0, 0, 0, 0],
                    )
                    if rest is not None:
                        for _ in rest:
                            pass

    return nc


def _prep_inputs(x, w1, w2, w3, g1, b1, m1, v1, g2, b2, m2, v2, g3, b3, m3, v3):
    """Fold BN into weights/shifts and pack everything into per-core maps."""

    def fold(wv, g, bb, m, v):
        inv = (g / np.sqrt(v + EPS)).astype(np.float32)
        shift = (bb - m * inv).astype(np.float32)
        return np.asarray(wv, np.float32) * inv[:, None, None, None], shift

    def to_f8(a):
        return np.clip(a, -240.0, 240.0).astype(F8_NP)

    w1f, sh1 = fold(np.asarray(w1, np.float32), g1, b1, m1, v1)
    w2f, sh2 = fold(np.asarray(w2, np.float32), g2, b2, m2, v2)
    w3f, sh3 = fold(np.asarray(w3, np.float32), g3, b3, m3, v3)

    # lhsT layouts: partition = K-within-chunk, free = [kc?, d?, mc, m]
    w1h = to_f8(np.ascontiguousarray(
        (B1 * w1f)[:, :, 0, 0].T.reshape(KC_IN, P, MC_W, P).transpose(1, 0, 2, 3)
    ))
    w2h = to_f8(np.ascontiguousarray(
        (B2 * w2f).transpose(2, 3, 1, 0)    # [ky, kx, in, out]
        .reshape(9, MC_W, P, MC_W, P)
        .transpose(2, 0, 1, 3, 4)
    ))
    w3h = to_f8(np.ascontiguousarray(
        ((B3 / A2) * w3f)[:, :, 0, 0].T
        .reshape(MC_W, P, MC_OUT, P).transpose(1, 0, 2, 3)
    ))
    s1h = np.ascontiguousarray((A1 * sh1).reshape(MC_W, P).T)
    s2h = np.ascontiguousarray((A2 * sh2).reshape(MC_W, P).T)
    s3h = np.ascontiguousarray(sh3.reshape(MC_OUT, P).T)
    s3bh = np.ascontiguousarray((B3 * sh3).reshape(MC_OUT, P).T)

    xf32 = np.asarray(x, np.float32)
    xnp = xf32.astype(BF16_NP)
    x8np = to_f8(AX * xf32)
    in_maps = []
    for c in range(NCORES):
        sl = slice(c * BPC, (c + 1) * BPC)
        xc = np.ascontiguousarray(
            xnp[sl].reshape(BPC, KC_IN, P, S).transpose(0, 2, 1, 3)
        )
        x8c = np.ascontiguousarray(
            x8np[sl].reshape(BPC, KC_IN, P, S).transpose(0, 2, 1, 3)
        )
        in_maps.append({
            "x": xc, "x8": x8c, "w1": w1h, "w2": w2h, "w3": w3h,
            "s1": s1h, "s2": s2h, "s3": s3h, "s3b": s3bh,
            "ident": (B3 * np.eye(P)).astype(BF16_NP),
        })
    return in_maps


def _ensure_ntff_hook():
    """If tracing is requested but this image's antenv lacks axon_hooks,
    register an in-process shim (or disable tracing) so run_bass_kernel_spmd
    doesn't crash on the import."""
    if os.environ.get("BASS_TRACE") != "1":
        return
    try:
        import antenv.axon_hooks  # noqa: F401
        return
    except ImportError:
        pass
    try:
        import sys
        import types
        import antenv
        from trn_agent_boot.trn_boot import _ntff_profile_via_ctypes

        hook = _ntff_profile_via_ctypes("/opt/axon/libaxon_pjrt.so")
        mod = types.ModuleType("antenv.axon_hooks")
        state = {"hook": hook}
        mod.set_axon_ntff_profile_hook = lambda h: state.__setitem__("hook", h)
        mod.get_axon_ntff_profile_hook = lambda: state["hook"]
        antenv.axon_hooks = mod
        sys.modules["antenv.axon_hooks"] = mod
    except Exception:
        os.environ["BASS_NEVER_TRACE"] = "1"


def kernel(**inputs):
    global LAST_RESULT
    _ensure_ntff_hook()
    if "nc" not in _NC_CACHE:
        nc = _build_nc()
        _split_multi_waits(nc)  # HW-only legalization; CoreSim can't run it
        _NC_CACHE["nc"] = nc
    nc = _NC_CACHE["nc"]
    in_maps = _prep_inputs(**inputs)
    res = run_bass_kernel_spmd(nc, in_maps, list(range(NCORES)))
    LAST_RESULT = res
    outs = []
    for r in res.results:
        of = r["o"].astype(np.float32)      # [BPC, MC_OUT, P, S]
        of[:, list(SCALED_MC)] /= B3        # undo the VectorE-path scaling
        outs.append(of)
    out = np.concatenate(outs, axis=0)
    return np.ascontiguousarray(out.reshape(B, COUT, H, W))
